# revision 12
# baseline (speedup 1.0000x reference)
"""Multi-head attention block (B=2, S=2048, D=1024, H=16) on 8 trn2 cores.

Sharding: core c = (batch b = c//4, head-group g = c%4); each core computes
4 heads of one batch (Megatron column-shard of wq/wk/wv, row-shard of wo,
combined with data-parallel over batch). Host sums the 4 partial outputs
per batch and adds the (folded) bias.

Device-side layout strategy (all matmul contractions live on the partition
axis, no on-chip transposes):
  - activations are fed pre-transposed: qT/kT/vT [D, S]
  - Q,K are projected directly transposed: QT/KT [256, S] (psum = wT.T @ qT)
  - V is projected in natural layout [S, 256] with a ones column appended
    per head (so P @ [V|1] yields both O and the softmax row-sums l)
  - scores are computed transposed: ST[j, i] = KT_h.T @ QT_h, so softmax'
    exp runs on ACT and the PV matmul consumes P without any transpose
  - softmax skips max-subtraction (scores here are O(1); exp is safe in f32)
  - normalization: r = 1/l (DVE), broadcast across partitions on GPSIMD,
    one DVE multiply
  - output projection computed transposed (partial^T [D, S]); host transposes
Matmuls run in float32r (full-rate fp32 path: 8-bit exp / 11-bit mantissa,
~1e-4 element precision). Walrus requires every producer of an f32r matmul
operand to itself write f32r, so all matmul-feeding tiles are allocated
float32r; host pre-rounds the DMA-fed arrays to the f32r grid.
"""

import numpy as np

import concourse.bass as bass
import concourse.mybir as mybir
import concourse.tile as tile
from concourse import bacc
from concourse.bass_utils import run_bass_kernel_spmd

B, S, D, H = 2, 2048, 1024, 16
DK = D // H                  # 64
NCORES = 8
GROUPS = NCORES // B         # 4 head-groups
HPC = H // GROUPS            # 4 heads per core
OL = HPC * DK                # 256 local features
SB = 512                     # query-block (i) width
JB = 128                     # key-block (j) width
NSB = S // SB                # 4
NJB = S // JB                # 16
VS = DK + 1                  # V columns per head incl. ones column (65)

F32 = mybir.dt.float32
F32R = mybir.dt.float32r

NEG = -1e9

LAST_RUN = None  # stash of BassKernelResults for test harness inspection


def _round_f32r(a):
    """Round an f32 array to the f32r grid (top-20-bit float, round to
    nearest) so the raw DMA'd bits are well-rounded f32r."""
    a = np.ascontiguousarray(a, np.float32)
    u = a.view(np.uint32)
    u = (u + 0x7FF + ((u >> 12) & 1)) & np.uint32(0xFFFFF000)
    return u.view(np.float32)


def _classify_mask(mask2):
    """Per (ib, jb) block schedule derived from the boolean mask [S, S]
    (mask2[i, j] True = visible).

    Returns (jlists, bias_tiles):
      jlists[ib] = list of (jb, sub_ops) for j-blocks with any visible entry,
        where sub_ops = list over the 4 i-subblocks (128 wide) of
        ('v', None) visible / ('m', None) fully masked / ('x', bias_idx).
      bias_tiles: [n, JB, 128] f32 additive bias (transposed: [j, i]).
    """
    jlists = []
    bias_tiles = []
    assert mask2.any(axis=1).all(), "mask has a fully-masked query row"
    for ib in range(NSB):
        jl = []
        for jb in range(NJB):
            sub = mask2[ib * SB:(ib + 1) * SB, jb * JB:(jb + 1) * JB]
            if not sub.any():
                continue
            sub_ops = []
            for k in range(SB // 128):
                s2 = sub[k * 128:(k + 1) * 128, :]
                if s2.all():
                    sub_ops.append(("v", None))
                elif not s2.any():
                    sub_ops.append(("m", None))
                else:
                    bias_tiles.append(
                        np.where(s2, np.float32(0), np.float32(NEG)).T
                    )
                    sub_ops.append(("x", len(bias_tiles) - 1))
            jl.append((jb, sub_ops))
        jlists.append(jl)
    return jlists, bias_tiles


def _exp_runs(sub_ops):
    """Contiguous runs of non-masked i-subblocks: list of (k0, k1)."""
    runs = []
    start = None
    for k, (st, _) in enumerate(sub_ops):
        if st == "m":
            if start is not None:
                runs.append((start, k))
                start = None
        elif start is None:
            start = k
    if start is not None:
        runs.append((start, len(sub_ops)))
    return runs


def _build(jlists, nbias):
    nc = bacc.Bacc()

    qT = nc.dram_tensor("qT", [D, S], F32R, kind="ExternalInput")
    kT = nc.dram_tensor("kT", [D, S], F32R, kind="ExternalInput")
    vT = nc.dram_tensor("vT", [D, S], F32R, kind="ExternalInput")
    wqT = nc.dram_tensor("wqT", [D, OL], F32R, kind="ExternalInput")
    wkT = nc.dram_tensor("wkT", [D, OL], F32R, kind="ExternalInput")
    wvT = nc.dram_tensor("wvT", [D, OL], F32R, kind="ExternalInput")
    woT = nc.dram_tensor("woT", [OL, D], F32R, kind="ExternalInput")
    bqd = nc.dram_tensor("bq", [OL, 1], F32, kind="ExternalInput")
    bkd = nc.dram_tensor("bk", [OL, 1], F32, kind="ExternalInput")
    onesd = nc.dram_tensor("ones4", [128, HPC, 1], F32R, kind="ExternalInput")
    if nbias:
        mbd = nc.dram_tensor("maskb", [nbias, JB, 128], F32,
                             kind="ExternalInput")
    out = nc.dram_tensor("out", [D, S], F32, kind="ExternalOutput")

    ND = D // 128  # 8 contraction blocks

    with tile.TileContext(nc) as tc:
        with tc.tile_pool(name="consts", bufs=1) as consts:
            # resident SBUF tensors (f32r: matmul operands)
            QT = [consts.tile([128, S], F32R, name=f"QT{t}") for t in range(2)]
            KT = [consts.tile([128, S], F32R, name=f"KT{t}") for t in range(2)]
            XT = [consts.tile([128, S], F32R, name=f"XT{t}") for t in range(2)]
            Vt = [consts.tile([128, HPC * VS], F32R, name=f"V{st}")
                  for st in range(S // 128)]
            wq_t = [consts.tile([128, OL], F32R, name=f"wq{d}")
                    for d in range(ND)]
            wk_t = [consts.tile([128, OL], F32R, name=f"wk{d}")
                    for d in range(ND)]
            wv_t = [consts.tile([128, OL], F32R, name=f"wv{d}")
                    for d in range(ND)]
            wo_t = [consts.tile([128, D], F32R, name=f"wo{t}")
                    for t in range(2)]
            bq_t = [consts.tile([128, 1], F32, name=f"bq{t}") for t in range(2)]
            bk_t = [consts.tile([128, 1], F32, name=f"bk{t}") for t in range(2)]
            mb = [consts.tile([JB, 128], F32, name=f"mb{i}")
                  for i in range(nbias)]

            for d in range(ND):
                nc.sync.dma_start(wq_t[d][:], wqT[d * 128:(d + 1) * 128, :])
                nc.sync.dma_start(wk_t[d][:], wkT[d * 128:(d + 1) * 128, :])
                nc.sync.dma_start(wv_t[d][:], wvT[d * 128:(d + 1) * 128, :])
            for t in range(2):
                nc.sync.dma_start(wo_t[t][:], woT[t * 128:(t + 1) * 128, :])
                nc.sync.dma_start(bq_t[t][:], bqd[t * 128:(t + 1) * 128, :])
                nc.sync.dma_start(bk_t[t][:], bkd[t * 128:(t + 1) * 128, :])
            for i in range(nbias):
                nc.sync.dma_start(mb[i][:], mbd[i])

            # ---------------- phase 1: projections ----------------
            with tc.tile_pool(name="acts", bufs=3) as actp, \
                 tc.tile_pool(name="ppj", bufs=3, space="PSUM") as ppj, \
                 tc.tile_pool(name="ppv", bufs=4, space="PSUM") as ppv:

                # QT / KT: psum[o_tile 128, s 512] = sum_d wT[d,o].T @ actT[d,s]
                for dst, wt, act, bias in ((QT, wq_t, qT, bq_t),
                                           (KT, wk_t, kT, bk_t)):
                    for sb in range(NSB):
                        ps = [ppj.tile([128, SB], F32, tag="pj", name="ps")
                              for _ in range(2)]
                        for d in range(ND):
                            at = actp.tile([128, SB], F32R, tag="act")
                            nc.sync.dma_start(
                                at[:],
                                act[d * 128:(d + 1) * 128,
                                    sb * SB:(sb + 1) * SB])
                            for ot in range(2):
                                nc.tensor.matmul(
                                    ps[ot][:],
                                    wt[d][:, ot * 128:(ot + 1) * 128],
                                    at[:],
                                    start=(d == 0), stop=(d == ND - 1))
                        for ot in range(2):
                            # psum -> sbuf (f32r) with per-partition bias
                            nc.vector.tensor_scalar_add(
                                dst[ot][:, sb * SB:(sb + 1) * SB],
                                ps[ot][:], bias[ot][:])

                # V: psum[s_tile 128, o 256] = vT[d, s].T @ wvT[d, o]
                for sb in range(NSB):
                    psv = [ppv.tile([128, OL], F32, tag="pv", name="psv")
                           for _ in range(4)]
                    for d in range(ND):
                        at = actp.tile([128, SB], F32R, tag="act")
                        nc.sync.dma_start(
                            at[:],
                            vT[d * 128:(d + 1) * 128, sb * SB:(sb + 1) * SB])
                        for k in range(4):
                            nc.tensor.matmul(
                                psv[k][:],
                                at[:, k * 128:(k + 1) * 128],
                                wv_t[d][:],
                                start=(d == 0), stop=(d == ND - 1))
                    for k in range(4):
                        st = sb * 4 + k
                        v3 = Vt[st][:].rearrange("p (h c) -> p h c", c=VS)
                        nc.vector.tensor_copy(
                            v3[:, :, 0:DK],
                            psv[k][:].rearrange("p (h c) -> p h c", c=DK))
                        nc.sync.dma_start(v3[:, :, DK:VS], onesd[:])

            # ---------------- phase 2: attention ----------------
            with tc.tile_pool(name="pss", bufs=4, space="PSUM") as pss, \
                 tc.tile_pool(name="pso", bufs=3, space="PSUM") as pso, \
                 tc.tile_pool(name="pP", bufs=6) as pP, \
                 tc.tile_pool(name="prr", bufs=2) as prr, \
                 tc.tile_pool(name="prc", bufs=2) as prc:
                for hp in range(HPC // 2):      # head pairs, packed on PE
                    hpair = (2 * hp, 2 * hp + 1)
                    for ib in range(NSB):
                        jl = jlists[ib]
                        Ops = [pso.tile([VS, SB], F32, tag="O", name="Ops")
                               for _ in hpair]
                        for idx, (jb, sub_ops) in enumerate(jl):
                            first, last = idx == 0, idx == len(jl) - 1
                            Ps = []
                            for z, h in enumerate(hpair):
                                bp = 64 * z
                                Sp = pss.tile([JB, SB], F32, tag="S",
                                              name="Sp")
                                nc.tensor.matmul(
                                    Sp[:],
                                    KT[hp][bp:bp + 64,
                                           jb * JB:(jb + 1) * JB],
                                    QT[hp][bp:bp + 64,
                                           ib * SB:(ib + 1) * SB],
                                    start=True, stop=True)
                                for k, (stt, bidx) in enumerate(sub_ops):
                                    if stt == "x":
                                        nc.vector.tensor_add(
                                            Sp[:, k * 128:(k + 1) * 128],
                                            Sp[:, k * 128:(k + 1) * 128],
                                            mb[bidx][:])
                                P = pP.tile([JB, SB], F32R, tag="P", name="P")
                                for k, (stt, _) in enumerate(sub_ops):
                                    if stt == "m":
                                        nc.vector.tensor_scalar_mul(
                                            P[:, k * 128:(k + 1) * 128],
                                            Sp[:, k * 128:(k + 1) * 128], 0.0)
                                for k0, k1 in _exp_runs(sub_ops):
                                    nc.scalar.activation(
                                        P[:, k0 * 128:k1 * 128],
                                        Sp[:, k0 * 128:k1 * 128],
                                        mybir.ActivationFunctionType.Exp)
                                Ps.append(P)
                            for z, h in enumerate(hpair):
                                nc.tensor.matmul(
                                    Ops[z][:],
                                    Vt[jb][:, VS * h:VS * h + VS],
                                    Ps[z][:],
                                    start=first, stop=last)
                        for z, h in enumerate(hpair):
                            bp = 64 * z
                            rr = prr.tile([1, SB], F32, tag="r", name="rr")
                            nc.vector.reciprocal(rr[:], Ops[z][DK:VS, :])
                            Rc = prc.tile([64, SB], F32, tag="rc", name="Rc")
                            nc.gpsimd.partition_broadcast(Rc[:], rr[:])
                            nc.vector.tensor_mul(
                                XT[hp][bp:bp + 64, ib * SB:(ib + 1) * SB],
                                Ops[z][0:DK, :], Rc[:])

            # ---------------- phase 3: output projection ----------------
            with tc.tile_pool(name="po", bufs=4, space="PSUM") as pout, \
                 tc.tile_pool(name="obuf", bufs=4) as outp:
                for jt in range(D // 128):
                    for sb in range(NSB):
                        ps = pout.tile([128, SB], F32, tag="po", name="pso2")
                        for ot in range(2):
                            nc.tensor.matmul(
                                ps[:],
                                wo_t[ot][:, jt * 128:(jt + 1) * 128],
                                XT[ot][:, sb * SB:(sb + 1) * SB],
                                start=(ot == 0), stop=(ot == 1))
                        ob = outp.tile([128, SB], F32, tag="ob", name="ob")
                        nc.vector.tensor_copy(ob[:], ps[:])
                        nc.sync.dma_start(
                            out[jt * 128:(jt + 1) * 128,
                                sb * SB:(sb + 1) * SB], ob[:])
    nc.finalize()
    return nc


def kernel(q, k, v, mask, wq, bq, wk, bk, wv, bv, wo, bo):
    global LAST_RUN
    q, k, v = (np.asarray(x, np.float32) for x in (q, k, v))
    wq, bq, wk, bk = (np.asarray(x, np.float32) for x in (wq, bq, wk, bk))
    wv, bv, wo, bo = (np.asarray(x, np.float32) for x in (wv, bv, wo, bo))
    mask2 = np.asarray(mask)[0, 0] != 0

    jlists, bias_tiles = _classify_mask(mask2)
    nbias = len(bias_tiles)
    maskb = (np.stack(bias_tiles).astype(np.float32)
             if nbias else None)

    scale = np.float32(1.0 / np.sqrt(DK))
    bo_eff = (bo + wo @ bv).astype(np.float32)

    # per-(batch) transposed activations, shared across the 4 group-cores
    qTs = [_round_f32r(q[b].T) for b in range(B)]
    kTs = [_round_f32r(k[b].T) for b in range(B)]
    vTs = [_round_f32r(v[b].T) for b in range(B)]

    # per-(group) weight shards
    wqTs, wkTs, wvTs, woTs, bqs, bks = [], [], [], [], [], []
    for g in range(GROUPS):
        rows = slice(g * OL, (g + 1) * OL)
        wqTs.append(_round_f32r((wq[rows] * scale).T))
        wkTs.append(_round_f32r(wk[rows].T))
        wvTs.append(_round_f32r(wv[rows].T))
        woTs.append(_round_f32r(wo[:, rows].T))
        bqs.append(np.ascontiguousarray((bq[rows] * scale)[:, None]))
        bks.append(np.ascontiguousarray(bk[rows][:, None]))

    in_maps = []
    for c in range(NCORES):
        b, g = c // GROUPS, c % GROUPS
        m = {
            "qT": qTs[b], "kT": kTs[b], "vT": vTs[b],
            "wqT": wqTs[g], "wkT": wkTs[g], "wvT": wvTs[g],
            "woT": woTs[g], "bq": bqs[g], "bk": bks[g],
            "ones4": np.ones((128, HPC, 1), np.float32),
        }
        if nbias:
            m["maskb"] = maskb
        in_maps.append(m)

    nc = _build(jlists, nbias)
    res = run_bass_kernel_spmd(nc, in_maps, core_ids=list(range(NCORES)))
    LAST_RUN = res
    if res.exec_time_ns is not None:
        print(f"HW exec time: {res.exec_time_ns} ns")

    outp = np.zeros((B, S, D), np.float32)
    for c in range(NCORES):
        b = c // GROUPS
        outp[b] += res.results[c]["out"].T
    outp += bo_eff
    return outp


# revision 21
# speedup vs baseline: 1.0970x; 1.0970x over previous
"""Multi-head attention block (B=2, S=2048, D=1024, H=16) on 8 trn2 cores.

Sharding: core c = (batch b = c//4, head-group g = c%4); each core computes
4 heads of one batch (Megatron column-shard of wq/wk/wv, row-shard of wo,
combined with data-parallel over batch). Host sums the 4 partial outputs
per batch and adds the (folded) bias.

Device-side layout strategy (all matmul contractions live on the partition
axis, no on-chip transposes):
  - activations are fed pre-transposed: qT/kT/vT [D, S]
  - Q,K are projected directly transposed: QT/KT [256, S] (psum = wT.T @ qT)
  - V is projected in natural layout [S, 256] with a ones column appended
    per head (so P @ [V|1] yields both O and the softmax row-sums l)
  - scores are computed transposed: ST[j, i] = KT_h.T @ QT_h, so softmax'
    exp runs on ACT and the PV matmul consumes P without any transpose
  - softmax skips max-subtraction (scores here are O(1); exp is safe in f32)
  - normalization: r = 1/l (DVE), broadcast across partitions on GPSIMD,
    one DVE multiply
  - output projection computed transposed (partial^T [D, S]); host transposes
Matmuls run in float32r (full-rate fp32 path: 8-bit exp / 11-bit mantissa,
~1e-4 element precision). Walrus requires every producer of an f32r matmul
operand to itself write f32r, so all matmul-feeding tiles are allocated
float32r; host pre-rounds the DMA-fed arrays to the f32r grid.
"""

import numpy as np

import concourse.bass as bass
import concourse.mybir as mybir
import concourse.tile as tile
from concourse import bacc
from concourse.bass_utils import run_bass_kernel_spmd

B, S, D, H = 2, 2048, 1024, 16
DK = D // H                  # 64
NCORES = 8
GROUPS = NCORES // B         # 4 head-groups
HPC = H // GROUPS            # 4 heads per core
OL = HPC * DK                # 256 local features
SB = 512                     # query-block (i) width
JB = 128                     # key-block (j) width
NSB = S // SB                # 4
NJB = S // JB                # 16
VS = DK + 1                  # V columns per head incl. ones column (65)

F32 = mybir.dt.float32
F32R = mybir.dt.float32r

NEG = -1e9

LAST_RUN = None  # stash of BassKernelResults for test harness inspection


def _round_f32r(a):
    """Round an f32 array to the f32r grid (top-20-bit float, round to
    nearest) so the raw DMA'd bits are well-rounded f32r."""
    a = np.ascontiguousarray(a, np.float32)
    u = a.view(np.uint32)
    u = (u + 0x7FF + ((u >> 12) & 1)) & np.uint32(0xFFFFF000)
    return u.view(np.float32)


def _classify_mask(mask2):
    """Per (ib, jb) block schedule derived from the boolean mask [S, S]
    (mask2[i, j] True = visible).

    Returns (jlists, bias_tiles):
      jlists[ib] = list of (jb, sub_ops) for j-blocks with any visible entry,
        where sub_ops = list over the 4 i-subblocks (128 wide) of
        ('v', None) visible / ('m', None) fully masked / ('x', bias_idx).
      bias_tiles: [n, JB, 128] f32 additive bias (transposed: [j, i]).
    """
    jlists = []
    bias_tiles = []
    assert mask2.any(axis=1).all(), "mask has a fully-masked query row"
    for ib in range(NSB):
        jl = []
        for jb in range(NJB):
            sub = mask2[ib * SB:(ib + 1) * SB, jb * JB:(jb + 1) * JB]
            if not sub.any():
                continue
            sub_ops = []
            for k in range(SB // 128):
                s2 = sub[k * 128:(k + 1) * 128, :]
                if s2.all():
                    sub_ops.append(("v", None))
                elif not s2.any():
                    sub_ops.append(("m", None))
                else:
                    bias_tiles.append(
                        np.where(s2, np.float32(0), np.float32(NEG)).T
                    )
                    sub_ops.append(("x", len(bias_tiles) - 1))
            jl.append((jb, sub_ops))
        jlists.append(jl)
    return jlists, bias_tiles


def _exp_runs(sub_ops):
    """Contiguous runs of non-masked i-subblocks: list of (k0, k1)."""
    runs = []
    start = None
    for k, (st, _) in enumerate(sub_ops):
        if st == "m":
            if start is not None:
                runs.append((start, k))
                start = None
        elif start is None:
            start = k
    if start is not None:
        runs.append((start, len(sub_ops)))
    return runs


def _build(jlists, nbias):
    nc = bacc.Bacc()

    qT = nc.dram_tensor("qT", [D, S], F32R, kind="ExternalInput")
    kT = nc.dram_tensor("kT", [D, S], F32R, kind="ExternalInput")
    vT = nc.dram_tensor("vT", [D, S], F32R, kind="ExternalInput")
    wqT = nc.dram_tensor("wqT", [D, OL], F32R, kind="ExternalInput")
    wkT = nc.dram_tensor("wkT", [D, OL], F32R, kind="ExternalInput")
    wvT = nc.dram_tensor("wvT", [D, OL], F32R, kind="ExternalInput")
    woT = nc.dram_tensor("woT", [OL, D], F32R, kind="ExternalInput")
    bqd = nc.dram_tensor("bq", [OL, 1], F32, kind="ExternalInput")
    bkd = nc.dram_tensor("bk", [OL, 1], F32, kind="ExternalInput")
    onesd = nc.dram_tensor("ones4", [128, HPC, 1], F32R, kind="ExternalInput")
    if nbias:
        mbd = nc.dram_tensor("maskb", [nbias, JB, 128], F32,
                             kind="ExternalInput")
    out = nc.dram_tensor("out", [D, S], F32, kind="ExternalOutput")

    ND = D // 128  # 8 contraction blocks

    with tile.TileContext(nc) as tc:
        with tc.tile_pool(name="consts", bufs=1) as consts:
            # resident SBUF tensors (f32r: matmul operands)
            QT = [consts.tile([128, S], F32R, name=f"QT{t}") for t in range(2)]
            KT = [consts.tile([128, S], F32R, name=f"KT{t}") for t in range(2)]
            XT = [consts.tile([128, S], F32R, name=f"XT{t}") for t in range(2)]
            Vt = [consts.tile([128, HPC * VS], F32R, name=f"V{st}")
                  for st in range(S // 128)]
            wq_t = [consts.tile([128, OL], F32R, name=f"wq{d}")
                    for d in range(ND)]
            wk_t = [consts.tile([128, OL], F32R, name=f"wk{d}")
                    for d in range(ND)]
            wv_t = [consts.tile([128, OL], F32R, name=f"wv{d}")
                    for d in range(ND)]
            wo_t = [consts.tile([128, D], F32R, name=f"wo{t}")
                    for t in range(2)]
            bq_t = [consts.tile([128, 1], F32, name=f"bq{t}") for t in range(2)]
            bk_t = [consts.tile([128, 1], F32, name=f"bk{t}") for t in range(2)]
            mb = [consts.tile([JB, 128], F32, name=f"mb{i}")
                  for i in range(nbias)]

            for t in range(2):
                nc.sync.dma_start(bq_t[t][:], bqd[t * 128:(t + 1) * 128, :])
                nc.sync.dma_start(bk_t[t][:], bkd[t * 128:(t + 1) * 128, :])

            # ---------------- phase 1: projections ----------------
            with tc.tile_pool(name="acts", bufs=6) as actp, \
                 tc.tile_pool(name="ppj", bufs=4, space="PSUM") as ppj, \
                 tc.tile_pool(name="ppv", bufs=4, space="PSUM") as ppv:

                # QT / KT: psum[o_tile 128, s 512] = sum_d wT[d,o].T @ actT[d,s]
                for dst, wt, wdram, act, bias in (
                        (QT, wq_t, wqT, qT, bq_t),
                        (KT, wk_t, wkT, kT, bk_t)):
                    for sb in range(NSB):
                        ps = [ppj.tile([128, SB], F32, tag="pj", name="ps")
                              for _ in range(2)]
                        for d in range(ND):
                            if sb == 0:
                                # interleave weight loads with the act
                                # stream so the first matmuls start early
                                nc.sync.dma_start(
                                    wt[d][:],
                                    wdram[d * 128:(d + 1) * 128, :])
                            at = actp.tile([128, SB], F32R, tag="act")
                            nc.sync.dma_start(
                                at[:],
                                act[d * 128:(d + 1) * 128,
                                    sb * SB:(sb + 1) * SB])
                            for ot in range(2):
                                nc.tensor.matmul(
                                    ps[ot][:],
                                    wt[d][:, ot * 128:(ot + 1) * 128],
                                    at[:],
                                    start=(d == 0), stop=(d == ND - 1))
                        for ot in range(2):
                            # psum -> sbuf (f32r) with per-partition bias
                            nc.vector.tensor_scalar_add(
                                dst[ot][:, sb * SB:(sb + 1) * SB],
                                ps[ot][:], bias[ot][:])

                # V: psum[s_tile 128, o 256] = vT[d, s].T @ wvT[d, o]
                for sb in range(NSB):
                    psv = [ppv.tile([128, OL], F32, tag="pv", name="psv")
                           for _ in range(4)]
                    for d in range(ND):
                        if sb == 0:
                            nc.sync.dma_start(
                                wv_t[d][:], wvT[d * 128:(d + 1) * 128, :])
                        at = actp.tile([128, SB], F32R, tag="act")
                        nc.sync.dma_start(
                            at[:],
                            vT[d * 128:(d + 1) * 128, sb * SB:(sb + 1) * SB])
                        for k in range(4):
                            nc.tensor.matmul(
                                psv[k][:],
                                at[:, k * 128:(k + 1) * 128],
                                wv_t[d][:],
                                start=(d == 0), stop=(d == ND - 1))
                    for k in range(4):
                        st = sb * 4 + k
                        v3 = Vt[st][:].rearrange("p (h c) -> p h c", c=VS)
                        nc.vector.tensor_copy(
                            v3[:, :, 0:DK],
                            psv[k][:].rearrange("p (h c) -> p h c", c=DK))
                        nc.sync.dma_start(v3[:, :, DK:VS], onesd[:])

            # mask bias tiles are first needed here; DMA them late so they
            # don't delay the projection-phase activation loads
            for i in range(nbias):
                nc.sync.dma_start(mb[i][:], mbd[i])

            # ---------------- phase 2: attention ----------------
            with tc.tile_pool(name="pss", bufs=4, space="PSUM") as pss, \
                 tc.tile_pool(name="pso", bufs=4, space="PSUM") as pso, \
                 tc.tile_pool(name="pP", bufs=6) as pP, \
                 tc.tile_pool(name="prr", bufs=2) as prr, \
                 tc.tile_pool(name="prc", bufs=2) as prc:
                for hp in range(HPC // 2):      # head pairs, packed on PE
                    hpair = (2 * hp, 2 * hp + 1)
                    for ib in range(NSB):
                        jl = jlists[ib]
                        Ops = [pso.tile([VS, SB], F32, tag="O", name="Ops")
                               for _ in hpair]

                        def emit_pv(pend):
                            jb_p, Ps_p, first, last = pend
                            for z, h in enumerate(hpair):
                                nc.tensor.matmul(
                                    Ops[z][:],
                                    Vt[jb_p][:, VS * h:VS * h + VS],
                                    Ps_p[z][:],
                                    start=first, stop=last)

                        # software-pipelined: PV for block j is emitted after
                        # the S/exp of block j+1, so the PE never waits on ACT
                        pending = None
                        for idx, (jb, sub_ops) in enumerate(jl):
                            Ps = []
                            for z, h in enumerate(hpair):
                                bp = 64 * z
                                Sp = pss.tile([JB, SB], F32, tag="S",
                                              name="Sp")
                                nc.tensor.matmul(
                                    Sp[:],
                                    KT[hp][bp:bp + 64,
                                           jb * JB:(jb + 1) * JB],
                                    QT[hp][bp:bp + 64,
                                           ib * SB:(ib + 1) * SB],
                                    start=True, stop=True)
                                for k, (stt, bidx) in enumerate(sub_ops):
                                    if stt == "x":
                                        nc.vector.tensor_add(
                                            Sp[:, k * 128:(k + 1) * 128],
                                            Sp[:, k * 128:(k + 1) * 128],
                                            mb[bidx][:])
                                P = pP.tile([JB, SB], F32R, tag="P", name="P")
                                for k, (stt, _) in enumerate(sub_ops):
                                    if stt == "m":
                                        nc.vector.tensor_scalar_mul(
                                            P[:, k * 128:(k + 1) * 128],
                                            Sp[:, k * 128:(k + 1) * 128], 0.0)
                                for k0, k1 in _exp_runs(sub_ops):
                                    nc.scalar.activation(
                                        P[:, k0 * 128:k1 * 128],
                                        Sp[:, k0 * 128:k1 * 128],
                                        mybir.ActivationFunctionType.Exp)
                                Ps.append(P)
                            if pending is not None:
                                emit_pv(pending)
                            pending = (jb, Ps, idx == 0, idx == len(jl) - 1)
                        emit_pv(pending)
                        for z, h in enumerate(hpair):
                            bp = 64 * z
                            rr = prr.tile([1, SB], F32, tag="r", name="rr")
                            nc.vector.reciprocal(rr[:], Ops[z][DK:VS, :])
                            Rc = prc.tile([64, SB], F32, tag="rc", name="Rc")
                            nc.gpsimd.partition_broadcast(Rc[:], rr[:])
                            nc.vector.tensor_mul(
                                XT[hp][bp:bp + 64, ib * SB:(ib + 1) * SB],
                                Ops[z][0:DK, :], Rc[:])

            # ---------------- phase 3: output projection ----------------
            for t in range(2):
                nc.sync.dma_start(wo_t[t][:], woT[t * 128:(t + 1) * 128, :])
            with tc.tile_pool(name="po", bufs=4, space="PSUM") as pout, \
                 tc.tile_pool(name="obuf", bufs=4) as outp:
                for jt in range(D // 128):
                    for sb in range(NSB):
                        ps = pout.tile([128, SB], F32, tag="po", name="pso2")
                        for ot in range(2):
                            nc.tensor.matmul(
                                ps[:],
                                wo_t[ot][:, jt * 128:(jt + 1) * 128],
                                XT[ot][:, sb * SB:(sb + 1) * SB],
                                start=(ot == 0), stop=(ot == 1))
                        ob = outp.tile([128, SB], F32, tag="ob", name="ob")
                        nc.vector.tensor_copy(ob[:], ps[:])
                        nc.sync.dma_start(
                            out[jt * 128:(jt + 1) * 128,
                                sb * SB:(sb + 1) * SB], ob[:])
    nc.finalize()
    return nc


def kernel(q, k, v, mask, wq, bq, wk, bk, wv, bv, wo, bo):
    global LAST_RUN
    q, k, v = (np.asarray(x, np.float32) for x in (q, k, v))
    wq, bq, wk, bk = (np.asarray(x, np.float32) for x in (wq, bq, wk, bk))
    wv, bv, wo, bo = (np.asarray(x, np.float32) for x in (wv, bv, wo, bo))
    mask2 = np.asarray(mask)[0, 0] != 0

    jlists, bias_tiles = _classify_mask(mask2)
    nbias = len(bias_tiles)
    maskb = (np.stack(bias_tiles).astype(np.float32)
             if nbias else None)

    scale = np.float32(1.0 / np.sqrt(DK))
    bo_eff = (bo + wo @ bv).astype(np.float32)

    # per-(batch) transposed activations, shared across the 4 group-cores
    qTs = [_round_f32r(q[b].T) for b in range(B)]
    kTs = [_round_f32r(k[b].T) for b in range(B)]
    vTs = [_round_f32r(v[b].T) for b in range(B)]

    # per-(group) weight shards
    wqTs, wkTs, wvTs, woTs, bqs, bks = [], [], [], [], [], []
    for g in range(GROUPS):
        rows = slice(g * OL, (g + 1) * OL)
        wqTs.append(_round_f32r((wq[rows] * scale).T))
        wkTs.append(_round_f32r(wk[rows].T))
        wvTs.append(_round_f32r(wv[rows].T))
        woTs.append(_round_f32r(wo[:, rows].T))
        bqs.append(np.ascontiguousarray((bq[rows] * scale)[:, None]))
        bks.append(np.ascontiguousarray(bk[rows][:, None]))

    in_maps = []
    for c in range(NCORES):
        b, g = c // GROUPS, c % GROUPS
        m = {
            "qT": qTs[b], "kT": kTs[b], "vT": vTs[b],
            "wqT": wqTs[g], "wkT": wkTs[g], "wvT": wvTs[g],
            "woT": woTs[g], "bq": bqs[g], "bk": bks[g],
            "ones4": np.ones((128, HPC, 1), np.float32),
        }
        if nbias:
            m["maskb"] = maskb
        in_maps.append(m)

    nc = _build(jlists, nbias)
    res = run_bass_kernel_spmd(nc, in_maps, core_ids=list(range(NCORES)))
    LAST_RUN = res
    if res.exec_time_ns is not None:
        print(f"HW exec time: {res.exec_time_ns} ns")

    outp = np.zeros((B, S, D), np.float32)
    for c in range(NCORES):
        b = c // GROUPS
        outp[b] += res.results[c]["out"].T
    outp += bo_eff
    return outp
